# revision 25
# baseline (speedup 1.0000x reference)
"""Expert-routed BERT attention (MoE top-1 over batch rows) on 8 Trainium2 cores.

Strategy
--------
Routing (mean over seq -> squared distance to 2 centers -> argmin) is a
per-batch-row decision on ~25K floats, computed on host while preparing the
shard inputs.  Each of the 8 cores processes 4 batch rows; for every row the
host gathers exactly the assigned expert's weights, so the device kernel is a
fully static dense pipeline (no on-device control flow, no wasted expert):

  per row r (feature-major activations xT [D,S]):
    QT/KT  = [Wq/sqrt(dh) | Wk]^T-projection  (K=128 dense matmuls, bias fused)
    V      = row-major x @ Wv, packed per head with an appended ones column
    per head h:
      scoresT[k,q] = KT_h^T-tiles @ QT_h        (K=64; even/odd heads placed on
                                                 partition halves 0-63/64-127 so
                                                 consecutive matmuls overlap in
                                                 disjoint PE row-groups)
      expT = exp(scoresT)  (ACT, psum->sbuf bf16; no max-subtraction needed:
                            |scores| <= ~6 for this problem scale)
      ctx2T[65,S] = [V_h | 1]^T @ expT          (row 64 = softmax denominator)
      denominators: reciprocal on DVE, broadcast across partitions via a
      DRAM round-trip DMA (step-0 partition reads are DRAM-only on this
      stack); odd heads' normalized context is DMA-moved to partitions
      64-127 so the output projection contracts K=128 head pairs.
    out[q,dout] = sum_hp ctxn_pair^T-tiles @ Wo_pair  (row-major out)

The kernel BIR is post-processed for this walrus build: sync waits are
capped at 1 per instruction (excess hoisted onto NoOps) and repeated
back-to-back Ldweights of the same stationary operand are elided.

Matmuls run in bf16 (fp32 PE matmul is 4x slower); accumulation is fp32 in
PSUM.  attention_mask is all-ones per the problem spec (fill=ones) and
bv/bo are zeros in setup_inputs; bq/bk are folded into the QK projection
bias adds (also zeros in practice).  Output is fp32.
"""

import numpy as np
import ml_dtypes

import concourse.bass as bass
import concourse.mybir as mybir
import concourse.tile as tile
from concourse.bass_utils import run_bass_kernel_spmd

F32 = mybir.dt.float32
BF16 = mybir.dt.bfloat16
ActFn = mybir.ActivationFunctionType

B, S, D, H, E = 32, 512, 768, 12, 2
DH = D // H            # 64
NCORES = 8
RPC = B // NCORES      # 4 rows per core
DC = D // 128          # 6 contraction chunks of 128
NT = S // 128          # 4 tiles of 128 along seq (q and k)
DHALF = D // 2         # 384 (psum-bank sized output slices)

_COMPILED_NC = None
LAST_RESULT = None     # BassKernelResults of the most recent run (for test.py)

_WSPLIT_CTR = [0]
# Wait-slot capacity by instruction class on this walrus build (probed):
# probed: CTRL-struct and TensorScalarPtr take 1; default to 1 for all.
_WAIT_CAPS = {"InstDrain": 1, "InstNoOp": 1}


def _split_excess_waits(nc, maxw=2):
    """This walrus build caps sync waits per instruction (1 for CTRL-struct,
    2 elsewhere).  Hoist excess waits onto injected same-engine NoOps —
    engines are in-order, so semantics are preserved."""
    nsplit = 0
    for f in nc.m.functions:
        for b in f.blocks:
            new = []
            for inst in list(b.instructions):
                si = getattr(inst, "sync_info", None)
                waits = list(si.on_wait) if si is not None and si.on_wait else []
                cap = _WAIT_CAPS.get(type(inst).__name__, maxw)
                if len(waits) > cap:
                    nop_cap = _WAIT_CAPS["InstNoOp"]
                    extra, keep = waits[:-cap], waits[-cap:]
                    for ci in range(0, len(extra), nop_cap):
                        _WSPLIT_CTR[0] += 1
                        nop = mybir.InstNoOp(
                            name=f"I-wsplit-{_WSPLIT_CTR[0]}",
                            engine=inst.engine,
                            ins=[],
                            outs=[],
                            sync_info=mybir.SyncInfo(
                                on_wait=extra[ci:ci + nop_cap], on_update=[]),
                        )
                        nc.register_instruction(nop, overwrite=True)
                        new.append(nop)
                    inst.sync_info = mybir.SyncInfo(
                        on_wait=keep,
                        on_update=list(si.on_update) if si.on_update else [])
                    nsplit += 1
                new.append(inst)
            b.instructions = new
    return nsplit


def _dedupe_ldweights(nc):
    """Walrus re-loads PE weights per matmul (ldw-opt unavailable for bass
    kernels); when consecutive PE matmuls share the same stationary operand,
    replace the repeated Ldweights with a sync-preserving NoOp."""
    ndrop = 0
    for f in nc.m.functions:
        for b in f.blocks:
            il = list(b.instructions)
            new = []
            last_ldw_key = None
            for inst in il:
                cls = type(inst).__name__
                if getattr(inst, "engine", None) == mybir.EngineType.PE:
                    if cls == "InstLdweights":
                        ap = inst.ins[0]
                        key = str(ap)
                        tp = getattr(inst, "tile_position", None)
                        key = (key, str(tp))
                        if key == last_ldw_key:
                            si = getattr(inst, "sync_info", None)
                            has_upd = si is not None and si.on_update
                            if not has_upd:
                                nop = mybir.InstNoOp(
                                    name=inst.name + "-ldwdup",
                                    engine=inst.engine,
                                    ins=[], outs=[],
                                    sync_info=si)
                                nc.register_instruction(nop, overwrite=True)
                                new.append(nop)
                                ndrop += 1
                                continue
                        last_ldw_key = key
                    elif cls not in ("InstMatmult", "InstNoOp"):
                        last_ldw_key = None
                new.append(inst)
            b.instructions = new
    return ndrop


def _build_nc():
    nc = bass.Bass()
    xt_d = nc.declare_dram_parameter("xt", [RPC, 128, DC, S], BF16, isOutput=False)
    wqk_d = nc.declare_dram_parameter("wqk", [RPC, 128, DC, 2 * D], BF16, isOutput=False)
    wv_d = nc.declare_dram_parameter("wv", [RPC, 128, DC, D], BF16, isOutput=False)
    wo_d = nc.declare_dram_parameter("wo", [RPC, 128, DC, D], BF16, isOutput=False)
    bqk_d = nc.declare_dram_parameter("bqk", [128, RPC * 2 * DC], F32, isOutput=False)
    out_d = nc.declare_dram_parameter("out", [RPC, NT, 128, D], F32, isOutput=True)

    with tile.TileContext(nc) as tc:
        with (
            tc.tile_pool(name="weights", bufs=2) as wpool,
            tc.tile_pool(name="weights1", bufs=1) as wpool1,
            tc.tile_pool(name="acts", bufs=2) as apool,
            tc.tile_pool(name="expp", bufs=12) as epool,
            tc.tile_pool(name="big1", bufs=4) as bpool,
            tc.tile_pool(name="big2", bufs=2) as b1pool,
            tc.tile_pool(name="norm", bufs=2) as npool,
            tc.tile_pool(name="outp", bufs=3) as opool,
            tc.tile_pool(name="psS", bufs=2, space="PSUM") as psS,
            tc.tile_pool(name="psC", bufs=2, space="PSUM") as psC,
            tc.tile_pool(name="psQ", bufs=2, space="PSUM") as psQ,
            tc.tile_pool(name="psO", bufs=2, space="PSUM") as psO,
            tc.tile_pool(name="scr", bufs=2, space="DRAM") as scrpool,
        ):
            bqk_sb = wpool1.tile([128, RPC * 2 * DC], F32, tag="bqk")
            nc.sync.dma_start(bqk_sb[:], bqk_d[:])

            for r in range(RPC):
                # ---- input loads (one contiguous DMA each) ----
                xt_sb = apool.tile([128, DC, S], BF16, tag="xt")
                nc.gpsimd.dma_start(xt_sb[:], xt_d[r])
                wv_sb = wpool1.tile([128, DC, D], BF16, tag="wv")
                nc.gpsimd.dma_start(wv_sb[:], wv_d[r])
                wqk_sb = wpool.tile([128, DC, 2 * D], BF16, tag="wqk")
                nc.gpsimd.dma_start(wqk_sb[:, :, 0:D], wqk_d[r, :, :, 0:D])
                nc.gpsimd.dma_start(wqk_sb[:, :, D:2 * D], wqk_d[r, :, :, D:2 * D])
                wo_sb = wpool1.tile([128, DC, D], BF16, tag="wo")
                nc.gpsimd.dma_start(wo_sb[:], wo_d[r])

                # ---- V projection: row-major [s, dout], head-grouped +ones ----
                vbuf = apool.tile([128, NT, H, DH + 1], BF16, tag="vbuf")
                for st in range(NT):
                    nc.vector.memset(vbuf[:, st, :, DH : DH + 1], 1.0)
                    psh = []
                    for half in range(2):
                        ps_t = psQ.tile([128, DHALF], F32, tag="proj")
                        psh.append(ps_t)
                    for k in range(DC):
                        for half in range(2):
                            nc.tensor.matmul(
                                psh[half][:],
                                xt_sb[:, k, st * 128:(st + 1) * 128],
                                wv_sb[:, k, half * DHALF:(half + 1) * DHALF],
                                start=(k == 0),
                                stop=(k == DC - 1),
                            )
                    for half in range(2):
                        nc.vector.tensor_copy(
                            vbuf[:, st, half * 6:(half + 1) * 6, 0:DH],
                            psh[half][:].rearrange("p (g d) -> p g d", d=DH),
                        )

                # ---- QT/KT projection: feature-major [dout, q] ----
                qkt_sb = apool.tile([128, 2 * DC, S], BF16, tag="qkt")
                for j in range(2 * DC):
                    ps = psQ.tile([128, S], F32, tag="proj")
                    for k in range(DC):
                        nc.tensor.matmul(
                            ps[:],
                            wqk_sb[:, k, j * 128:(j + 1) * 128],
                            xt_sb[:, k, :],
                            start=(k == 0),
                            stop=(k == DC - 1),
                        )
                    nc.vector.tensor_scalar_add(
                        qkt_sb[:, j, :], ps[:], bqk_sb[:, r * 2 * DC + j : r * 2 * DC + j + 1]
                    )

                # ---- attention, head pairs on partition halves ----
                # Per-pair ctxu/ctxn tiles keep Tile dependencies pair-local
                # (3D-sliced shared tiles serialize whole-tile), so each pair's
                # normalize chain overlaps the remaining attention work.
                ctxn = []
                for hp in range(DC):  # 6 head pairs
                    expt = [[None] * NT for _ in range(2)]  # [h2][kt]
                    for kt in range(NT):
                        for h2 in range(2):
                            base = h2 * DH
                            sc_t = psS.tile([128, S], F32, tag="scores")
                            nc.tensor.matmul(
                                sc_t[:],
                                qkt_sb[base:base + DH, DC + hp, kt * 128:(kt + 1) * 128],
                                qkt_sb[base:base + DH, hp, :],
                                tile_position=(base, 0),
                            )
                            et = epool.tile([128, S], BF16, tag="expt")
                            nc.scalar.activation(et[:], sc_t[:], ActFn.Exp)
                            expt[h2][kt] = et
                    ctxu = bpool.tile([DH + 1, 2, S], BF16, tag="ctxu")
                    for h2 in range(2):
                        h = 2 * hp + h2
                        ps_c = psC.tile([DH + 1, S], F32, tag="ctx")
                        for kt in range(NT):
                            nc.tensor.matmul(
                                ps_c[:],
                                vbuf[:, kt, h, :],
                                expt[h2][kt][:],
                                start=(kt == 0),
                                stop=(kt == NT - 1),
                            )
                        nc.vector.tensor_copy(ctxu[:, h2, :], ps_c[:])
                    dn = npool.tile([16, S // 8], BF16, tag="dn")
                    nc.sync.dma_start(dn[:], ctxu[DH : DH + 1, :, :])
                    rcp = npool.tile([16, S // 8], F32, tag="rcp")
                    nc.vector.reciprocal(rcp[:], dn[:])
                    scr = scrpool.tile([1, 2, S], F32, tag="scr")
                    nc.sync.dma_start(scr[:], rcp[:])
                    rb = npool.tile([DH, 2, S], F32, tag="rb")
                    nc.sync.dma_start(rb[:], scr[:, :, :].to_broadcast((DH, 2, S)))
                    cn = b1pool.tile([128, S], BF16, tag=f"ctxn{hp}")
                    stg = npool.tile([DH, S], BF16, tag="stg")
                    nc.vector.tensor_mul(stg[:], ctxu[0:DH, 1, :], rb[:, 1, :])
                    nc.sync.dma_start(cn[DH:128, :], stg[:])
                    nc.vector.tensor_mul(cn[0:DH, :], ctxu[0:DH, 0, :], rb[:, 0, :])
                    ctxn.append(cn)

                # ---- output projection: row-major [q, dout] ----
                # Split the head-chunk accumulation: chunks 0-2 are ready
                # mid-attention and accumulate early; chunks 3-5 close late and
                # are combined with a DVE add, keeping PE busy during the last
                # pairs' normalize chains.
                for qt in range(NT):
                    out_sb = opool.tile([128, D], F32, tag="osb")
                    psA = []
                    for half in range(2):
                        psA_t = psO.tile([128, DHALF], F32, tag="oproj")
                        psA.append(psA_t)
                    for hp in range(DC // 2):
                        for half in range(2):
                            nc.tensor.matmul(
                                psA[half][:],
                                ctxn[hp][:, qt * 128:(qt + 1) * 128],
                                wo_sb[:, hp, half * DHALF:(half + 1) * DHALF],
                                start=(hp == 0),
                                stop=(hp == DC // 2 - 1),
                            )
                    for half in range(2):
                        nc.vector.tensor_copy(out_sb[:, half * DHALF:(half + 1) * DHALF], psA[half][:])
                    psB = []
                    for half in range(2):
                        psB_t = psO.tile([128, DHALF], F32, tag="oproj")
                        psB.append(psB_t)
                    for hpo in range(DC // 2):
                        hp = DC // 2 + hpo
                        for half in range(2):
                            nc.tensor.matmul(
                                psB[half][:],
                                ctxn[hp][:, qt * 128:(qt + 1) * 128],
                                wo_sb[:, hp, half * DHALF:(half + 1) * DHALF],
                                start=(hpo == 0),
                                stop=(hpo == DC // 2 - 1),
                            )
                    for half in range(2):
                        nc.vector.tensor_add(
                            out_sb[:, half * DHALF:(half + 1) * DHALF],
                            out_sb[:, half * DHALF:(half + 1) * DHALF],
                            psB[half][:])
                        nc.gpsimd.dma_start(
                            out_d[r, qt, :, half * DHALF:(half + 1) * DHALF],
                            out_sb[:, half * DHALF:(half + 1) * DHALF])

    _dedupe_ldweights(nc)
    _split_excess_waits(nc, maxw=1)
    nc.finalize()
    return nc


def _get_nc():
    global _COMPILED_NC
    if _COMPILED_NC is None:
        _COMPILED_NC = _build_nc()
    return _COMPILED_NC


def _prep_expert_tables(centers, Wq, bq, Wk, bk, Wv, bv, Wo, bo):
    """Per-expert packed weight tables in the DRAM layouts the kernel expects."""
    scale = 1.0 / np.sqrt(np.float32(DH))
    bf16 = ml_dtypes.bfloat16
    wqk_e, wv_e, wo_e, bqk_e = [], [], [], []
    for e in range(E):
        wqk = np.concatenate([Wq[e] * scale, Wk[e]], axis=1)          # [D, 2D]
        wqk_e.append(np.ascontiguousarray(
            wqk.reshape(DC, 128, 2 * D).transpose(1, 0, 2)).astype(bf16))  # [128, DC, 2D]
        wv_e.append(np.ascontiguousarray(
            Wv[e].reshape(DC, 128, D).transpose(1, 0, 2)).astype(bf16))    # [128, DC, D]
        wo_e.append(np.ascontiguousarray(
            Wo[e].reshape(DC, 128, D).transpose(1, 0, 2)).astype(bf16))    # [128, DC, D]
        bqk = np.concatenate([bq[e] * scale, bk[e]])                   # [2D]
        bqk_e.append(np.ascontiguousarray(
            bqk.reshape(2 * DC, 128).T).astype(np.float32))            # [128, 2*DC]
    return wqk_e, wv_e, wo_e, bqk_e


def _ensure_axon_hooks():
    """bass_utils imports antenv.axon_hooks when BASS_TRACE is set under axon;
    provide a no-op registry if this environment lacks the module."""
    try:
        import antenv.axon_hooks  # noqa: F401
        return
    except ImportError:
        pass
    import sys
    import types
    try:
        import antenv
    except ImportError:
        return
    mod = types.ModuleType("antenv.axon_hooks")
    mod._hook = None
    mod.set_axon_ntff_profile_hook = lambda h: setattr(mod, "_hook", h)
    mod.get_axon_ntff_profile_hook = lambda: mod._hook
    try:
        import os
        from trn_agent_boot.trn_boot import _ntff_profile_via_ctypes
        so = "/opt/axon/libaxon_pjrt.so"
        if os.path.exists(so):
            mod.set_axon_ntff_profile_hook(_ntff_profile_via_ctypes(so))
    except Exception:
        pass
    sys.modules["antenv.axon_hooks"] = mod
    antenv.axon_hooks = mod


def kernel(hidden_states, attention_mask, centers, Wq, bq, Wk, bk, Wv, bv, Wo, bo):
    hs = np.asarray(hidden_states, dtype=np.float32)
    mask = np.asarray(attention_mask, dtype=np.float32)
    centers = np.asarray(centers, dtype=np.float32)
    Wq, bq = np.asarray(Wq, np.float32), np.asarray(bq, np.float32)
    Wk, bk = np.asarray(Wk, np.float32), np.asarray(bk, np.float32)
    Wv, bv = np.asarray(Wv, np.float32), np.asarray(bv, np.float32)
    Wo, bo = np.asarray(Wo, np.float32), np.asarray(bo, np.float32)

    # Structural assumptions from the problem spec (fill=ones mask, zero biases
    # that have no per-partition slot on-device).
    assert np.all(mask == 1.0), "kernel assumes all-ones attention_mask"
    assert not bv.any() and not bo.any(), "kernel assumes zero bv/bo"

    # ---- routing on host (tiny): mean over seq -> nearest center ----
    hmean = hs.mean(axis=1)                                            # [B, D]
    d2 = ((hmean[:, None, :] - centers[None, :, :]) ** 2).sum(-1)      # [B, E]
    assign = d2.argmin(axis=1)                                         # [B]

    wqk_e, wv_e, wo_e, bqk_e = _prep_expert_tables(
        centers, Wq, bq, Wk, bk, Wv, bv, Wo, bo)

    bf16 = ml_dtypes.bfloat16
    in_maps = []
    for c in range(NCORES):
        rows = list(range(c * RPC, (c + 1) * RPC))
        xt = np.stack([
            np.ascontiguousarray(hs[b].T.reshape(DC, 128, S).transpose(1, 0, 2))
            for b in rows]).astype(bf16)                               # [RPC, 128, DC, S]
        in_maps.append({
            "xt": xt,
            "wqk": np.stack([wqk_e[assign[b]] for b in rows]),
            "wv": np.stack([wv_e[assign[b]] for b in rows]),
            "wo": np.stack([wo_e[assign[b]] for b in rows]),
            "bqk": np.concatenate([bqk_e[assign[b]] for b in rows], axis=1),
        })

    _ensure_axon_hooks()
    global LAST_RESULT
    LAST_RESULT = run_bass_kernel_spmd(_get_nc(), in_maps, list(range(NCORES)))

    out = np.empty((B, S, D), dtype=np.float32)
    for c in range(NCORES):
        o = LAST_RESULT.results[c]["out"]                              # [RPC, NT, 128, D]
        for r in range(RPC):
            out[c * RPC + r] = np.asarray(o[r], np.float32).reshape(S, D)
    return out


# revision 26
# speedup vs baseline: 1.0596x; 1.0596x over previous
"""Expert-routed BERT attention (MoE top-1 over batch rows) on 8 Trainium2 cores.

Strategy
--------
Routing (mean over seq -> squared distance to 2 centers -> argmin) is a
per-batch-row decision on ~25K floats, computed on host while preparing the
shard inputs.  Each of the 8 cores processes 4 batch rows; for every row the
host gathers exactly the assigned expert's weights, so the device kernel is a
fully static dense pipeline (no on-device control flow, no wasted expert):

  per row r (feature-major activations xT [D,S]):
    QT/KT  = [Wq/sqrt(dh) | Wk]^T-projection  (K=128 dense matmuls, bias fused)
    V      = row-major x @ Wv, packed per head with an appended ones column
    per head h:
      scoresT[k,q] = KT_h^T-tiles @ QT_h        (K=64; even/odd heads placed on
                                                 partition halves 0-63/64-127 so
                                                 consecutive matmuls overlap in
                                                 disjoint PE row-groups)
      expT = exp(scoresT)  (ACT, psum->sbuf bf16; no max-subtraction needed:
                            |scores| <= ~6 for this problem scale)
      ctx2T[65,S] = [V_h | 1]^T @ expT          (row 64 = softmax denominator)
      denominators: reciprocal on DVE, broadcast across partitions via a
      DRAM round-trip DMA (step-0 partition reads are DRAM-only on this
      stack); odd heads' normalized context is DMA-moved to partitions
      64-127 so the output projection contracts K=128 head pairs.
    out[q,dout] = sum_hp ctxn_pair^T-tiles @ Wo_pair  (row-major out)

The kernel BIR is post-processed for this walrus build: sync waits are
capped at 1 per instruction (excess hoisted onto NoOps) and repeated
back-to-back Ldweights of the same stationary operand are elided.

Matmuls run in bf16 (fp32 PE matmul is 4x slower); accumulation is fp32 in
PSUM.  attention_mask is all-ones per the problem spec (fill=ones) and
bv/bo are zeros in setup_inputs; bq/bk are folded into the QK projection
bias adds (also zeros in practice).  Output is fp32.
"""

import numpy as np
import ml_dtypes

import concourse.bass as bass
import concourse.mybir as mybir
import concourse.tile as tile
from concourse.bass_utils import run_bass_kernel_spmd

F32 = mybir.dt.float32
BF16 = mybir.dt.bfloat16
ActFn = mybir.ActivationFunctionType

B, S, D, H, E = 32, 512, 768, 12, 2
DH = D // H            # 64
NCORES = 8
RPC = B // NCORES      # 4 rows per core
DC = D // 128          # 6 contraction chunks of 128
NT = S // 128          # 4 tiles of 128 along seq (q and k)
DHALF = D // 2         # 384 (psum-bank sized output slices)

_COMPILED_NC = None
LAST_RESULT = None     # BassKernelResults of the most recent run (for test.py)

_WSPLIT_CTR = [0]
# Wait-slot capacity by instruction class on this walrus build (probed):
# probed: CTRL-struct and TensorScalarPtr take 1; default to 1 for all.
_WAIT_CAPS = {"InstDrain": 1, "InstNoOp": 1}


def _split_excess_waits(nc, maxw=2):
    """This walrus build caps sync waits per instruction (1 for CTRL-struct,
    2 elsewhere).  Hoist excess waits onto injected same-engine NoOps —
    engines are in-order, so semantics are preserved."""
    nsplit = 0
    for f in nc.m.functions:
        for b in f.blocks:
            new = []
            for inst in list(b.instructions):
                si = getattr(inst, "sync_info", None)
                waits = list(si.on_wait) if si is not None and si.on_wait else []
                cap = _WAIT_CAPS.get(type(inst).__name__, maxw)
                if len(waits) > cap:
                    nop_cap = _WAIT_CAPS["InstNoOp"]
                    extra, keep = waits[:-cap], waits[-cap:]
                    for ci in range(0, len(extra), nop_cap):
                        _WSPLIT_CTR[0] += 1
                        nop = mybir.InstNoOp(
                            name=f"I-wsplit-{_WSPLIT_CTR[0]}",
                            engine=inst.engine,
                            ins=[],
                            outs=[],
                            sync_info=mybir.SyncInfo(
                                on_wait=extra[ci:ci + nop_cap], on_update=[]),
                        )
                        nc.register_instruction(nop, overwrite=True)
                        new.append(nop)
                    inst.sync_info = mybir.SyncInfo(
                        on_wait=keep,
                        on_update=list(si.on_update) if si.on_update else [])
                    nsplit += 1
                new.append(inst)
            b.instructions = new
    return nsplit


def _dedupe_ldweights(nc):
    """Walrus re-loads PE weights per matmul (ldw-opt unavailable for bass
    kernels); when consecutive PE matmuls share the same stationary operand,
    replace the repeated Ldweights with a sync-preserving NoOp."""
    ndrop = 0
    for f in nc.m.functions:
        for b in f.blocks:
            il = list(b.instructions)
            new = []
            last_ldw_key = None
            for inst in il:
                cls = type(inst).__name__
                if getattr(inst, "engine", None) == mybir.EngineType.PE:
                    if cls == "InstLdweights":
                        ap = inst.ins[0]
                        key = str(ap)
                        tp = getattr(inst, "tile_position", None)
                        key = (key, str(tp))
                        if key == last_ldw_key:
                            si = getattr(inst, "sync_info", None)
                            has_upd = si is not None and si.on_update
                            if not has_upd:
                                nop = mybir.InstNoOp(
                                    name=inst.name + "-ldwdup",
                                    engine=inst.engine,
                                    ins=[], outs=[],
                                    sync_info=si)
                                nc.register_instruction(nop, overwrite=True)
                                new.append(nop)
                                ndrop += 1
                                continue
                        last_ldw_key = key
                    elif cls not in ("InstMatmult", "InstNoOp"):
                        last_ldw_key = None
                new.append(inst)
            b.instructions = new
    return ndrop


def _build_nc():
    nc = bass.Bass()
    xt_d = nc.declare_dram_parameter("xt", [RPC, 128, DC, S], BF16, isOutput=False)
    wqk_d = nc.declare_dram_parameter("wqk", [RPC, 128, DC, 2 * D], BF16, isOutput=False)
    wv_d = nc.declare_dram_parameter("wv", [RPC, 128, DC, D], BF16, isOutput=False)
    wo_d = nc.declare_dram_parameter("wo", [RPC, 128, DC, D], BF16, isOutput=False)
    bqk_d = nc.declare_dram_parameter("bqk", [128, RPC * 2 * DC], F32, isOutput=False)
    out_d = nc.declare_dram_parameter("out", [RPC, NT, 128, D], F32, isOutput=True)

    with tile.TileContext(nc) as tc:
        with (
            tc.tile_pool(name="weights", bufs=2) as wpool,
            tc.tile_pool(name="weights1", bufs=1) as wpool1,
            tc.tile_pool(name="acts", bufs=2) as apool,
            tc.tile_pool(name="expp", bufs=8) as epool,
            tc.tile_pool(name="big1", bufs=4) as bpool,
            tc.tile_pool(name="big2", bufs=2) as b1pool,
            tc.tile_pool(name="norm", bufs=2) as npool,
            tc.tile_pool(name="outp", bufs=3) as opool,
            tc.tile_pool(name="psS", bufs=2, space="PSUM") as psS,
            tc.tile_pool(name="psC", bufs=2, space="PSUM") as psC,
            tc.tile_pool(name="psQ", bufs=2, space="PSUM") as psQ,
            tc.tile_pool(name="psO", bufs=2, space="PSUM") as psO,
            tc.tile_pool(name="scr", bufs=2, space="DRAM") as scrpool,
        ):
            bqk_sb = wpool1.tile([128, RPC * 2 * DC], F32, tag="bqk")
            nc.sync.dma_start(bqk_sb[:], bqk_d[:])

            for r in range(RPC):
                # ---- input loads (one contiguous DMA each) ----
                xt_sb = apool.tile([128, DC, S], BF16, tag="xt")
                nc.sync.dma_start(xt_sb[:], xt_d[r])
                wv_sb = wpool1.tile([128, DC, D], BF16, tag="wv")
                nc.sync.dma_start(wv_sb[:], wv_d[r])
                wqk_sb = wpool.tile([128, DC, 2 * D], BF16, tag="wqk")
                nc.sync.dma_start(wqk_sb[:, :, 0:D], wqk_d[r, :, :, 0:D])
                nc.sync.dma_start(wqk_sb[:, :, D:2 * D], wqk_d[r, :, :, D:2 * D])
                wo_sb = wpool1.tile([128, DC, D], BF16, tag="wo")
                nc.sync.dma_start(wo_sb[:], wo_d[r])

                # ---- V projection: row-major [s, dout], head-grouped +ones ----
                vbuf = apool.tile([128, NT, H, DH + 1], BF16, tag="vbuf")
                for st in range(NT):
                    nc.vector.memset(vbuf[:, st, :, DH : DH + 1], 1.0)
                    psh = []
                    for half in range(2):
                        ps_t = psQ.tile([128, DHALF], F32, tag="proj")
                        psh.append(ps_t)
                    for k in range(DC):
                        for half in range(2):
                            nc.tensor.matmul(
                                psh[half][:],
                                xt_sb[:, k, st * 128:(st + 1) * 128],
                                wv_sb[:, k, half * DHALF:(half + 1) * DHALF],
                                start=(k == 0),
                                stop=(k == DC - 1),
                            )
                    for half in range(2):
                        nc.vector.tensor_copy(
                            vbuf[:, st, half * 6:(half + 1) * 6, 0:DH],
                            psh[half][:].rearrange("p (g d) -> p g d", d=DH),
                        )

                # ---- QT/KT projection: feature-major [dout, q] ----
                qkt_sb = apool.tile([128, 2 * DC, S], BF16, tag="qkt")
                for j in range(2 * DC):
                    ps = psQ.tile([128, S], F32, tag="proj")
                    for k in range(DC):
                        nc.tensor.matmul(
                            ps[:],
                            wqk_sb[:, k, j * 128:(j + 1) * 128],
                            xt_sb[:, k, :],
                            start=(k == 0),
                            stop=(k == DC - 1),
                        )
                    nc.vector.tensor_scalar_add(
                        qkt_sb[:, j, :], ps[:], bqk_sb[:, r * 2 * DC + j : r * 2 * DC + j + 1]
                    )

                # ---- attention, head pairs on partition halves ----
                # Per-pair ctxu/ctxn tiles keep Tile dependencies pair-local
                # (3D-sliced shared tiles serialize whole-tile), so each pair's
                # normalize chain overlaps the remaining attention work.
                ctxn = []
                for hp in range(DC):  # 6 head pairs
                    expt = [[None] * NT for _ in range(2)]  # [h2][kt]
                    for kt in range(NT):
                        for h2 in range(2):
                            base = h2 * DH
                            sc_t = psS.tile([128, S], F32, tag="scores")
                            nc.tensor.matmul(
                                sc_t[:],
                                qkt_sb[base:base + DH, DC + hp, kt * 128:(kt + 1) * 128],
                                qkt_sb[base:base + DH, hp, :],
                                tile_position=(base, 0),
                            )
                            et = epool.tile([128, S], BF16, tag="expt")
                            nc.scalar.activation(et[:], sc_t[:], ActFn.Exp)
                            expt[h2][kt] = et
                    ctxu = bpool.tile([DH + 1, 2, S], BF16, tag="ctxu")
                    for h2 in range(2):
                        h = 2 * hp + h2
                        ps_c = psC.tile([DH + 1, S], F32, tag="ctx")
                        for kt in range(NT):
                            nc.tensor.matmul(
                                ps_c[:],
                                vbuf[:, kt, h, :],
                                expt[h2][kt][:],
                                start=(kt == 0),
                                stop=(kt == NT - 1),
                            )
                        nc.vector.tensor_copy(ctxu[:, h2, :], ps_c[:])
                    dn = npool.tile([16, S // 8], BF16, tag="dn")
                    nc.sync.dma_start(dn[:], ctxu[DH : DH + 1, :, :])
                    rcp = npool.tile([16, S // 8], F32, tag="rcp")
                    nc.vector.reciprocal(rcp[:], dn[:])
                    scr = scrpool.tile([1, 2, S], F32, tag="scr")
                    nc.sync.dma_start(scr[:], rcp[:])
                    rb = npool.tile([DH, 2, S], F32, tag="rb")
                    nc.sync.dma_start(rb[:], scr[:, :, :].to_broadcast((DH, 2, S)))
                    cn = b1pool.tile([128, S], BF16, tag=f"ctxn{hp}")
                    stg = npool.tile([DH, S], BF16, tag="stg")
                    nc.vector.tensor_mul(stg[:], ctxu[0:DH, 1, :], rb[:, 1, :])
                    nc.sync.dma_start(cn[DH:128, :], stg[:])
                    nc.vector.tensor_mul(cn[0:DH, :], ctxu[0:DH, 0, :], rb[:, 0, :])
                    ctxn.append(cn)

                # ---- output projection: row-major [q, dout] ----
                # Split the head-chunk accumulation: chunks 0-2 are ready
                # mid-attention and accumulate early; chunks 3-5 close late and
                # are combined with a DVE add, keeping PE busy during the last
                # pairs' normalize chains.
                for qt in range(NT):
                    out_sb = opool.tile([128, D], F32, tag="osb")
                    psA = []
                    for half in range(2):
                        psA_t = psO.tile([128, DHALF], F32, tag="oproj")
                        psA.append(psA_t)
                    for hp in range(DC // 2):
                        for half in range(2):
                            nc.tensor.matmul(
                                psA[half][:],
                                ctxn[hp][:, qt * 128:(qt + 1) * 128],
                                wo_sb[:, hp, half * DHALF:(half + 1) * DHALF],
                                start=(hp == 0),
                                stop=(hp == DC // 2 - 1),
                            )
                    for half in range(2):
                        nc.vector.tensor_copy(out_sb[:, half * DHALF:(half + 1) * DHALF], psA[half][:])
                    psB = []
                    for half in range(2):
                        psB_t = psO.tile([128, DHALF], F32, tag="oproj")
                        psB.append(psB_t)
                    for hpo in range(DC // 2):
                        hp = DC // 2 + hpo
                        for half in range(2):
                            nc.tensor.matmul(
                                psB[half][:],
                                ctxn[hp][:, qt * 128:(qt + 1) * 128],
                                wo_sb[:, hp, half * DHALF:(half + 1) * DHALF],
                                start=(hpo == 0),
                                stop=(hpo == DC // 2 - 1),
                            )
                    for half in range(2):
                        nc.vector.tensor_add(
                            out_sb[:, half * DHALF:(half + 1) * DHALF],
                            out_sb[:, half * DHALF:(half + 1) * DHALF],
                            psB[half][:])
                        nc.sync.dma_start(
                            out_d[r, qt, :, half * DHALF:(half + 1) * DHALF],
                            out_sb[:, half * DHALF:(half + 1) * DHALF])

    _dedupe_ldweights(nc)
    _split_excess_waits(nc, maxw=1)
    nc.finalize()
    return nc


def _get_nc():
    global _COMPILED_NC
    if _COMPILED_NC is None:
        _COMPILED_NC = _build_nc()
    return _COMPILED_NC


def _prep_expert_tables(centers, Wq, bq, Wk, bk, Wv, bv, Wo, bo):
    """Per-expert packed weight tables in the DRAM layouts the kernel expects."""
    scale = 1.0 / np.sqrt(np.float32(DH))
    bf16 = ml_dtypes.bfloat16
    wqk_e, wv_e, wo_e, bqk_e = [], [], [], []
    for e in range(E):
        wqk = np.concatenate([Wq[e] * scale, Wk[e]], axis=1)          # [D, 2D]
        wqk_e.append(np.ascontiguousarray(
            wqk.reshape(DC, 128, 2 * D).transpose(1, 0, 2)).astype(bf16))  # [128, DC, 2D]
        wv_e.append(np.ascontiguousarray(
            Wv[e].reshape(DC, 128, D).transpose(1, 0, 2)).astype(bf16))    # [128, DC, D]
        wo_e.append(np.ascontiguousarray(
            Wo[e].reshape(DC, 128, D).transpose(1, 0, 2)).astype(bf16))    # [128, DC, D]
        bqk = np.concatenate([bq[e] * scale, bk[e]])                   # [2D]
        bqk_e.append(np.ascontiguousarray(
            bqk.reshape(2 * DC, 128).T).astype(np.float32))            # [128, 2*DC]
    return wqk_e, wv_e, wo_e, bqk_e


def _ensure_axon_hooks():
    """bass_utils imports antenv.axon_hooks when BASS_TRACE is set under axon;
    provide a no-op registry if this environment lacks the module."""
    try:
        import antenv.axon_hooks  # noqa: F401
        return
    except ImportError:
        pass
    import sys
    import types
    try:
        import antenv
    except ImportError:
        return
    mod = types.ModuleType("antenv.axon_hooks")
    mod._hook = None
    mod.set_axon_ntff_profile_hook = lambda h: setattr(mod, "_hook", h)
    mod.get_axon_ntff_profile_hook = lambda: mod._hook
    try:
        import os
        from trn_agent_boot.trn_boot import _ntff_profile_via_ctypes
        so = "/opt/axon/libaxon_pjrt.so"
        if os.path.exists(so):
            mod.set_axon_ntff_profile_hook(_ntff_profile_via_ctypes(so))
    except Exception:
        pass
    sys.modules["antenv.axon_hooks"] = mod
    antenv.axon_hooks = mod


def kernel(hidden_states, attention_mask, centers, Wq, bq, Wk, bk, Wv, bv, Wo, bo):
    hs = np.asarray(hidden_states, dtype=np.float32)
    mask = np.asarray(attention_mask, dtype=np.float32)
    centers = np.asarray(centers, dtype=np.float32)
    Wq, bq = np.asarray(Wq, np.float32), np.asarray(bq, np.float32)
    Wk, bk = np.asarray(Wk, np.float32), np.asarray(bk, np.float32)
    Wv, bv = np.asarray(Wv, np.float32), np.asarray(bv, np.float32)
    Wo, bo = np.asarray(Wo, np.float32), np.asarray(bo, np.float32)

    # Structural assumptions from the problem spec (fill=ones mask, zero biases
    # that have no per-partition slot on-device).
    assert np.all(mask == 1.0), "kernel assumes all-ones attention_mask"
    assert not bv.any() and not bo.any(), "kernel assumes zero bv/bo"

    # ---- routing on host (tiny): mean over seq -> nearest center ----
    hmean = hs.mean(axis=1)                                            # [B, D]
    d2 = ((hmean[:, None, :] - centers[None, :, :]) ** 2).sum(-1)      # [B, E]
    assign = d2.argmin(axis=1)                                         # [B]

    wqk_e, wv_e, wo_e, bqk_e = _prep_expert_tables(
        centers, Wq, bq, Wk, bk, Wv, bv, Wo, bo)

    bf16 = ml_dtypes.bfloat16
    in_maps = []
    for c in range(NCORES):
        rows = list(range(c * RPC, (c + 1) * RPC))
        xt = np.stack([
            np.ascontiguousarray(hs[b].T.reshape(DC, 128, S).transpose(1, 0, 2))
            for b in rows]).astype(bf16)                               # [RPC, 128, DC, S]
        in_maps.append({
            "xt": xt,
            "wqk": np.stack([wqk_e[assign[b]] for b in rows]),
            "wv": np.stack([wv_e[assign[b]] for b in rows]),
            "wo": np.stack([wo_e[assign[b]] for b in rows]),
            "bqk": np.concatenate([bqk_e[assign[b]] for b in rows], axis=1),
        })

    _ensure_axon_hooks()
    global LAST_RESULT
    LAST_RESULT = run_bass_kernel_spmd(_get_nc(), in_maps, list(range(NCORES)))

    out = np.empty((B, S, D), dtype=np.float32)
    for c in range(NCORES):
        o = LAST_RESULT.results[c]["out"]                              # [RPC, NT, 128, D]
        for r in range(RPC):
            out[c * RPC + r] = np.asarray(o[r], np.float32).reshape(S, D)
    return out
